# revision 55
# baseline (speedup 1.0000x reference)
"""Single-head attention kernel for Trainium2 (Bass/Tile), 8 NeuronCores.

Problem: B=4, S=4096, D=1024, H=128 fp32.
    q,k,v = x @ W{q,k,v};  out = softmax(q k^T / sqrt(H)) @ v

Sharding: 8 cores = (batch b, KEY-half kh).  Each core computes PARTIAL
attention for all 4096 queries over its 2048 keys; the host combines the
two partial results per batch: out = (outT_0 + outT_1) / (l_0 + l_1).
The host permutes each core's x rows so its key rows come first and
lays xT out slice-major so every (partition, 512-row slice) is one
contiguous 8KB run -> 128 large DMA descriptors per slice instead of
1024 small ones.

fp16 everywhere on the matmul operands (fp8 fails the 2e-2 gate: the
max-error element sits in concentrated-attention rows where out ~= one
v row and quantization error doesn't average).  Matmul outputs are
512-col pieces: walrus rejects outputs crossing a PSUM bank
(s3d3_mm_num_elements), so 512 fp32 per matmul is an ISA limit.

Schedule shape (per core): the PE is the global bottleneck (~91us of
matmul columns) with the Activation engine's exp stream (~71us) second,
so the pre-attention front is cut to the true minimum (qt blocks 0-1 +
kt block 0) and ALL other projections are interleaved into the
attention chunks; chunk 0 lags its AV accumulation by 3 kb so even the
v0 chain can hide there.  Input DMA is split across the two HW DGE
queues (SP + Activation) with slice 0 halved across both, and the host
lays xT out so every (partition, slice) is one contiguous 8KB run.
PE-clock warmup runs on a memset tile so it needs no DMA.

Softmax denominators: attnT tiles are tree-summed on the DVE (pairs ->
quads -> one oct for kb 0-7) and the PE ones-matmul consumes one tile
per group: oct(0-7), quad(8-11), pair(12-13), at14, at15 - the last two
DIRECTLY so the in-order PE queue never waits on the DVE chain at a
chunk boundary (5120 columns/chunk instead of 8192).
Outputs are fp16 (host upcasts); outT lives in per-half PSUM banks so
each half's evacuation overlaps the other half's matmuls; l drains via
DVE mid-stream and via the Activation engine on the last chunk.
"""

import math

import numpy as np

import concourse.bacc as bacc
import concourse.mybir as mybir
import concourse.tile as tile
from concourse.bass_utils import run_bass_kernel_spmd

B, S, D, H = 4, 4096, 1024, 128
NCORES = 8
SK = S // 2  # keys per core (2048)
RB = 512  # rows per projection block
NRB = S // RB  # 8 blocks
NKRB = SK // RB  # 4 key blocks
QC = 1024  # queries per attention chunk
NQC = S // QC  # 4 chunks
NKB = SK // 128  # 16 key blocks of 128
NDC = D // 128  # 8 contraction chunks

F32 = mybir.dt.float32
F16 = mybir.dt.float16

_CACHE = {}


def build_nc():
    nc = bacc.Bacc("TRN2", target_bir_lowering=False, debug=False)

    # x^T, host-prearranged: xt[p, ((g*NDC + c)*RB + s)] = x[g*RB+s, c*128+p]
    xt_d = nc.dram_tensor("xt", [128, NRB * NDC * RB], F16, kind="ExternalInput")
    # weights split so the small wq part lands first and the q0 projection
    # starts sooner: wqpack = wq chunks 0-7; wrest = wk 0-7, wv 8-15,
    # 16 ident, 17 col0 = ones
    wqpack_d = nc.dram_tensor("wqpack", [128, 8 * 128], F16, kind="ExternalInput")
    wrest_d = nc.dram_tensor("wrest", [128, 18 * 128], F16, kind="ExternalInput")
    # partial (key-shard) unnormalized out^T [h, q] and denominators l [1, q]
    outT_d = nc.dram_tensor("outT", [H, S], F16, kind="ExternalOutput")
    l_d = nc.dram_tensor("l", [1, S], F16, kind="ExternalOutput")

    scale = 1.0 / math.sqrt(H)

    with tile.TileContext(nc) as tc:
        with (
            tc.tile_pool(name="const", bufs=1) as constp,
            tc.tile_pool(name="persist", bufs=1) as persist,
            tc.tile_pool(name="attn", bufs=8) as attn_pool,
            tc.tile_pool(name="fin", bufs=2) as fin_pool,
            tc.tile_pool(name="ps_p", bufs=1, space="PSUM") as ps_p,
            tc.tile_pool(name="ps_s", bufs=2, space="PSUM") as ps_s,
            tc.tile_pool(name="ps_o", bufs=1, space="PSUM") as ps_o,
        ):
            # ---- PE clock warmup on a memset tile: no DMA dependency ----
            warm = constp.tile([128, 128], F16, name="warm_sb")
            nc.vector.memset(warm[:], 0.0)
            warm_f32 = constp.tile([1, 1], F32, name="warm_f32")
            # also preloads the exp table on the Activation engine
            nc.scalar.activation(
                warm_f32[:], warm[0:1, 0:1], mybir.ActivationFunctionType.Exp
            )
            warm_ps = ps_p.tile([128, 128], F32, tag="proj")
            for i in range(64):
                nc.tensor.matmul(
                    warm_ps[:], warm[:], warm[:], start=(i == 0), stop=(i == 63)
                )

            # ---- input DMA, split across both HW DGE queues ----
            wqpack_sb = constp.tile([128, 8, 128], F16, name="wqpack_sb")
            wrest_sb = constp.tile([128, 18, 128], F16, name="wrest_sb")
            nc.scalar.dma_start(
                wqpack_sb[:], wqpack_d.ap().rearrange("p (c h) -> p c h", c=8)
            )

            xt_sb = persist.tile([128, NRB, NDC, RB], F16, name="xt_sb")

            def load_slice(eng, g, c0=0, c1=NDC):
                eng.dma_start(
                    xt_sb[:, g, c0:c1, :],
                    xt_d.ap()[
                        :, (g * NDC + c0) * RB : (g * NDC + c1) * RB
                    ].rearrange("p (c s) -> p c s", c=c1 - c0),
                )

            # slice 0 in halves split ACROSS the two queues (they complete in
            # parallel, so the first projection starts ~2us sooner); remaining
            # slices interleaved so each arrives well before its consumer
            load_slice(nc.sync, 0, 0, NDC // 2)
            load_slice(nc.scalar, 0, NDC // 2, NDC)
            nc.scalar.dma_start(
                wrest_sb[:], wrest_d.ap().rearrange("p (c h) -> p c h", c=18)
            )
            load_slice(nc.sync, 1)
            load_slice(nc.scalar, 2)
            load_slice(nc.sync, 3)
            load_slice(nc.scalar, 4)
            load_slice(nc.sync, 5)
            load_slice(nc.scalar, 6)
            load_slice(nc.sync, 7)

            def w(name, dc):
                if name == "wq":
                    return wqpack_sb[:, dc, :]
                return wrest_sb[:, (0 if name == "wk" else 8) + dc, :]

            ident = wrest_sb[:, 16, :]
            ones = wrest_sb[:, 17, 0:1]

            # ---- persistent activations ----
            qt_sb = persist.tile([128, S], F16, name="qt_sb")  # [h, q] all q
            kt_sb = persist.tile([128, SK], F16, name="kt_sb")  # [h, k] own
            v_sb = persist.tile([128, NKB, H], F16, name="v_sb")  # own keys
            vt_sb = persist.tile([128, SK], F16, name="vt_sb")  # staging

            def project(wname, dst_sb, rb, pool, tag, width):
                """One 512-row projection block through one PSUM bank."""
                ps = pool.tile([128, width], F32, tag=tag)
                for dc in range(NDC):
                    nc.tensor.matmul(
                        ps[:, 0:RB],
                        w(wname, dc),
                        xt_sb[:, rb, dc, :],
                        start=(dc == 0),
                        stop=(dc == NDC - 1),
                    )
                nc.vector.tensor_copy(dst_sb[:, rb * RB : (rb + 1) * RB], ps[:, 0:RB])

            def v_transpose(g):
                v_ps = ps_p.tile([128, RB], F16, tag="proj")
                for s in range(4):
                    nc.tensor.transpose(
                        v_ps[:, s * 128 : (s + 1) * 128],
                        vt_sb[:, g * RB + s * 128 : g * RB + (s + 1) * 128],
                        ident,
                    )
                nc.vector.tensor_copy(
                    v_sb[:, g * 4 : (g + 1) * 4, :].rearrange("p a b -> p (a b)"),
                    v_ps[:, 0 : 4 * H],
                )

            # Front: only what attention chunk 0 needs immediately,
            # accumulated in parallel on idle attention PSUM slots.
            # Minimal front: score(0) only needs qt blocks 0,1 and kt block 0.
            # Everything else (v0 included) is interleaved into the attention
            # chunks, whose exp stream then starts ~4us earlier; chunk 0 lags
            # its AV accumulation by 3 kb to give the v0 chain time.
            project("wq", qt_sb, 0, ps_s, "st", QC)
            project("wq", qt_sb, 1, ps_o, "l", 512)
            project("wk", kt_sb, 0, ps_s, "st", QC)

            # deferred[qcidx][kb] emitted after score(kb).
            deferred = {qc: {} for qc in range(NQC)}

            def defer(qc, kb, fn):
                deferred[qc].setdefault(kb, []).append(fn)

            # chunk 0: kt block g lands ~3 kb before score(4g); transposes
            # spaced so each ps_p item's predecessor has already evacuated
            # when the in-order PE queue reaches it (av lag 6 gives the v
            # chain the needed slack)
            defer(0, 0, lambda: project("wv", vt_sb, 0, ps_o, "outT0", 512))
            defer(0, 1, lambda: project("wk", kt_sb, 1, ps_p, "proj", RB))
            defer(0, 2, lambda: project("wv", vt_sb, 1, ps_p, "proj", RB))
            defer(0, 3, lambda: project("wk", kt_sb, 2, ps_p, "proj", RB))
            defer(0, 4, lambda: v_transpose(0))
            defer(0, 5, lambda: project("wv", vt_sb, 2, ps_p, "proj", RB))
            defer(0, 6, lambda: project("wk", kt_sb, 3, ps_p, "proj", RB))
            defer(0, 7, lambda: v_transpose(1))
            defer(0, 8, lambda: project("wv", vt_sb, 3, ps_p, "proj", RB))
            defer(0, 9, lambda: v_transpose(2))
            defer(0, 10, lambda: v_transpose(3))
            # qt blocks 2-7, needed one chunk ahead
            defer(0, 11, lambda: project("wq", qt_sb, 2, ps_p, "proj", RB))
            defer(0, 13, lambda: project("wq", qt_sb, 3, ps_p, "proj", RB))
            defer(1, 2, lambda: project("wq", qt_sb, 4, ps_p, "proj", RB))
            defer(1, 8, lambda: project("wq", qt_sb, 5, ps_p, "proj", RB))
            defer(2, 2, lambda: project("wq", qt_sb, 6, ps_p, "proj", RB))
            defer(2, 8, lambda: project("wq", qt_sb, 7, ps_p, "proj", RB))

            # ---- attention (software-pipelined by one kb) ----
            for qcidx in range(NQC):
                # per-half PSUM tiles: the h0 evacuation only waits on h0's
                # accumulation, overlapping the h1 matmuls at chunk end.
                # Created lazily at the first av so the chunk-0 v0-projection
                # (which borrows the outT0 bank) precedes them in rotation.
                outT_lazy = []

                def outT_ps(h):
                    if not outT_lazy:
                        outT_lazy.append(
                            ps_o.tile([128, 512], F32, tag="outT0", name="outT_ps0")
                        )
                        outT_lazy.append(
                            ps_o.tile([128, 512], F32, tag="outT1", name="outT_ps1")
                        )
                    return outT_lazy[h]
                # both 512-halves of l packed into ONE psum bank (partition 0
                # and partition 32 via tile_position col 32)
                l_ps = ps_o.tile([64, 512], F32, tag="l")
                at_tiles = {}
                pair_tiles = {}
                quad_tiles = {}

                def score(kb):
                    st_ps = ps_s.tile([128, QC], F32, tag="st")
                    for h in range(QC // 512):
                        nc.tensor.matmul(
                            st_ps[:, h * 512 : (h + 1) * 512],
                            kt_sb[:, kb * 128 : (kb + 1) * 128],
                            qt_sb[
                                :, qcidx * QC + h * 512 : qcidx * QC + (h + 1) * 512
                            ],
                            start=True,
                            stop=True,
                        )
                    at_sb = attn_pool.tile([128, QC], F16, tag="at")
                    nc.scalar.activation(
                        at_sb[:],
                        st_ps[:],
                        mybir.ActivationFunctionType.Exp,
                        scale=scale,
                    )
                    at_tiles[kb] = at_sb

                def accum_av(kb):
                    at_sb = at_tiles[kb]
                    for h in range(QC // 512):
                        nc.tensor.matmul(
                            outT_ps(h)[:],
                            v_sb[:, kb, :],
                            at_sb[:, h * 512 : (h + 1) * 512],
                            start=(kb == 0),
                            stop=(kb == NKB - 1),
                        )

                def pair_add(p):
                    pair = attn_pool.tile([128, QC], F16, tag="pair", bufs=3)
                    nc.vector.tensor_add(pair[:], at_tiles[2 * p][:], at_tiles[2 * p + 1][:])
                    pair_tiles[p] = pair

                def quad_add(qd):
                    quad = attn_pool.tile([128, QC], F16, tag="quad", bufs=3)
                    nc.vector.tensor_add(
                        quad[:], pair_tiles.pop(2 * qd)[:], pair_tiles.pop(2 * qd + 1)[:]
                    )
                    quad_tiles[qd] = quad

                def tree_add(dst, a, b):
                    t_ = attn_pool.tile([128, QC], F16, tag="lsum", bufs=3, name="lsum")
                    nc.vector.tensor_add(t_[:], a, b)
                    quad_tiles[dst] = t_

                # softmax-denominator matmul parts: the DVE tree folds
                # kb 0-13 into one "dec" tile; kb 14/15 feed the ones-matmul
                # DIRECTLY (dep = exp output only), and all three parts are
                # emitted after av15 so the in-order PE queue never waits on
                # the DVE chain
                NPART = 3

                def accum_l(part, src):
                    for h in range(QC // 512):
                        nc.tensor.matmul(
                            l_ps[h * 32 : h * 32 + 1, :],
                            ones,
                            src[:, h * 512 : (h + 1) * 512],
                            start=(part == 0),
                            stop=(part == NPART - 1),
                            tile_position=(0, h * 32),
                        )

                LAG = 6 if qcidx == 0 else 1
                score(0)
                for fn in deferred[qcidx].get(0, []):
                    fn()
                for kb in range(1, NKB):
                    score(kb)
                    for fn in deferred[qcidx].get(kb, []):
                        fn()
                    if kb % 2 == 1 and kb < 15:
                        pair_add((kb - 1) // 2)
                    if kb in (5, 9, 13):
                        quad_add((kb - 5) // 4)
                    if kb == 10:
                        tree_add("oct", quad_tiles.pop(0)[:], quad_tiles.pop(1)[:])
                    if kb == 14:
                        tree_add("hex", quad_tiles.pop("oct")[:], quad_tiles.pop(2)[:])
                    if kb == 15:
                        tree_add("dec", quad_tiles.pop("hex")[:], pair_tiles.pop(6)[:])
                    if kb >= LAG:
                        accum_av(kb - LAG)
                for j in range(NKB - LAG, NKB):
                    accum_av(j)
                accum_l(0, quad_tiles.pop("dec")[:])
                accum_l(1, at_tiles[14][:])
                accum_l(2, at_tiles[15][:])

                # evacuate: l via DVE copies mid-stream (the Activation
                # engine paces the attention phase), but via the Activation
                # engine on the last chunk so l and outT drain in parallel
                def evac_l():
                    l_sb = fin_pool.tile([1, QC], F16, tag="l_sb", name="l_sb")
                    if qcidx == NQC - 1:
                        nc.scalar.copy(l_sb[:, 0:512], l_ps[0:1, :])
                        nc.scalar.copy(l_sb[:, 512:1024], l_ps[32:33, :])
                    else:
                        nc.vector.tensor_copy(l_sb[:, 0:512], l_ps[0:1, :])
                        nc.vector.tensor_copy(l_sb[:, 512:1024], l_ps[32:33, :])
                    leng = nc.scalar if qcidx == NQC - 1 else nc.sync
                    leng.dma_start(
                        l_d.ap()[:, qcidx * QC : (qcidx + 1) * QC], l_sb[:]
                    )

                if qcidx != NQC - 1:
                    evac_l()
                for hh in range(2):
                    outT_sb = fin_pool.tile([128, 512], F16, tag="outT_sb", bufs=2)
                    if qcidx == NQC - 1 and hh == 1:
                        # last chunk: h1 copy on the (now idle) Activation
                        # engine, in parallel with the DVE h0 copy
                        nc.scalar.copy(outT_sb[:], outT_ps(hh)[:])
                    else:
                        nc.vector.tensor_copy(outT_sb[:], outT_ps(hh)[:])
                    oeng = nc.scalar if (qcidx == NQC - 1 and hh == 1) else nc.sync
                    oeng.dma_start(
                        outT_d.ap()[
                            :, qcidx * QC + hh * 512 : qcidx * QC + (hh + 1) * 512
                        ],
                        outT_sb[:],
                    )
                if qcidx == NQC - 1:
                    evac_l()

    nc.compile()
    return nc


def _get_nc():
    if "nc" not in _CACHE:
        _CACHE["nc"] = build_nc()
    return _CACHE["nc"]


def make_in_maps(inputs, Wq, Wk, Wv):
    inputs = np.asarray(inputs, dtype=np.float32)

    def _swz(wmat):
        # chunk c holds rows c*128..c*128+127: out[p, c, h] = W[c*128+p, h]
        w16 = np.asarray(wmat, dtype=np.float16)  # [D, H]
        return w16.reshape(NDC, 128, H).transpose(1, 0, 2)

    wqpack = np.ascontiguousarray(_swz(Wq).reshape(128, 8 * 128))
    wrest = np.zeros((128, 18, 128), dtype=np.float16)
    wrest[:, 0:8, :] = _swz(Wk)
    wrest[:, 8:16, :] = _swz(Wv)
    wrest[:, 16, :] = np.eye(128, dtype=np.float16)
    wrest[:, 17, 0] = 1.0
    wrest_flat = np.ascontiguousarray(wrest.reshape(128, 18 * 128))

    in_maps = []
    for c in range(NCORES):
        b, kh = divmod(c, 2)
        xb = inputs[b]
        # own key-half rows first; queries follow the same permutation
        xk = np.concatenate(
            [xb[kh * SK : (kh + 1) * SK], xb[(1 - kh) * SK : (2 - kh) * SK]], axis=0
        )
        x16 = xk.astype(np.float16)  # [S, D]
        # xt[p, g, c, s] = xk[g*RB+s, c*128+p]: slice-major, 8KB contiguous
        # per (partition, slice)
        xt = np.ascontiguousarray(
            x16.reshape(NRB, RB, NDC, 128).transpose(3, 0, 2, 1)
        ).reshape(128, NRB * NDC * RB)
        in_maps.append({"xt": xt, "wqpack": wqpack, "wrest": wrest_flat})
    return in_maps


def kernel(inputs, Wq, Wk, Wv):
    nc = _get_nc()
    in_maps = make_in_maps(inputs, Wq, Wk, Wv)

    res = run_bass_kernel_spmd(nc, in_maps, core_ids=list(range(NCORES)))

    out = np.empty((B, S, H), dtype=np.float32)
    for b in range(B):
        num = np.zeros((H, S), dtype=np.float32)
        den = np.zeros((1, S), dtype=np.float32)
        for kh in range(2):
            c = 2 * b + kh
            outT = res.results[c]["outT"].astype(np.float32)  # [H, S], permuted q
            l = res.results[c]["l"].astype(np.float32)  # [1, S]
            # queries were ordered [kh-half, other-half]; map back
            perm = np.concatenate(
                [
                    np.arange(kh * SK, (kh + 1) * SK),
                    np.arange((1 - kh) * SK, (2 - kh) * SK),
                ]
            )
            num[:, perm] += outT
            den[:, perm] += l
        out[b] = (num / den).T
    return out


# revision 57
# speedup vs baseline: 1.0354x; 1.0354x over previous
"""Single-head attention kernel for Trainium2 (Bass/Tile), 8 NeuronCores.

Problem: B=4, S=4096, D=1024, H=128 fp32.
    q,k,v = x @ W{q,k,v};  out = softmax(q k^T / sqrt(H)) @ v

Sharding: 8 cores = (batch b, KEY-half kh).  Each core computes PARTIAL
attention for all 4096 queries over its 2048 keys; the host combines the
two partial results per batch: out = (outT_0 + outT_1) / (l_0 + l_1).
The host permutes each core's x rows so its key rows come first and
lays xT out slice-major so every (partition, 512-row slice) is one
contiguous 8KB run -> 128 large DMA descriptors per slice instead of
1024 small ones.

fp16 everywhere on the matmul operands (fp8 fails the 2e-2 gate: the
max-error element sits in concentrated-attention rows where out ~= one
v row and quantization error doesn't average).

Schedule shape (per core): the PE is the global bottleneck (~91us of
matmul columns) with the Activation engine's exp stream (~71us) second,
so the pre-attention front is cut to the true minimum (qt blocks 0-1 +
kt block 0) and ALL other projections are interleaved into the
attention chunks; chunk 0 lags its AV accumulation by 3 kb so even the
v0 chain can hide there.  Input DMA is split across the two HW DGE
queues (SP + Activation) with slice 0 halved across both, and the host
lays xT out so every (partition, slice) is one contiguous 8KB run.
PE-clock warmup runs on a memset tile so it needs no DMA.

Softmax denominators: attnT tiles are tree-summed on the DVE (pairs ->
quads -> one oct for kb 0-7) and the PE ones-matmul consumes one tile
per group: oct(0-7), quad(8-11), pair(12-13), at14, at15 - the last two
DIRECTLY so the in-order PE queue never waits on the DVE chain at a
chunk boundary (5120 columns/chunk instead of 8192).
Outputs are fp16 (host upcasts); outT lives in per-half PSUM banks so
each half's evacuation overlaps the other half's matmuls; l drains via
DVE mid-stream and via the Activation engine on the last chunk.
"""

import math

import numpy as np

import concourse.bacc as bacc
import concourse.mybir as mybir
import concourse.tile as tile
from concourse.bass_utils import run_bass_kernel_spmd

B, S, D, H = 4, 4096, 1024, 128
NCORES = 8
SK = S // 2  # keys per core (2048)
RB = 512  # rows per projection block
NRB = S // RB  # 8 blocks
NKRB = SK // RB  # 4 key blocks
QC = 1024  # queries per attention chunk
NQC = S // QC  # 4 chunks
NKB = SK // 128  # 16 key blocks of 128
NDC = D // 128  # 8 contraction chunks

F32 = mybir.dt.float32
F16 = mybir.dt.float16

_CACHE = {}


def build_nc():
    nc = bacc.Bacc("TRN2", target_bir_lowering=False, debug=False)

    # x^T, host-prearranged: xt[p, ((g*NDC + c)*RB + s)] = x[g*RB+s, c*128+p]
    xt_d = nc.dram_tensor("xt", [128, NRB * NDC * RB], F16, kind="ExternalInput")
    # weights split so the small wq part lands first and the q0 projection
    # starts sooner: wqpack = wq chunks 0-7; wrest = wk 0-7, wv 8-15,
    # 16 ident, 17 col0 = ones
    wqpack_d = nc.dram_tensor("wqpack", [128, 8 * 128], F16, kind="ExternalInput")
    wrest_d = nc.dram_tensor("wrest", [128, 18 * 128], F16, kind="ExternalInput")
    # partial (key-shard) unnormalized out^T [h, q] and denominators l [1, q]
    outT_d = nc.dram_tensor("outT", [H, S], F16, kind="ExternalOutput")
    l_d = nc.dram_tensor("l", [1, S], F16, kind="ExternalOutput")

    scale = 1.0 / math.sqrt(H)

    with tile.TileContext(nc) as tc:
        with (
            tc.tile_pool(name="const", bufs=1) as constp,
            tc.tile_pool(name="persist", bufs=1) as persist,
            tc.tile_pool(name="attn", bufs=7) as attn_pool,
            tc.tile_pool(name="fin", bufs=2) as fin_pool,
            tc.tile_pool(name="ps_p", bufs=1, space="PSUM") as ps_p,
            tc.tile_pool(name="ps_s", bufs=2, space="PSUM") as ps_s,
            tc.tile_pool(name="ps_o", bufs=1, space="PSUM") as ps_o,
        ):
            # ---- PE clock warmup on a memset tile: no DMA dependency ----
            warm = constp.tile([128, 128], F16, name="warm_sb")
            nc.vector.memset(warm[:], 0.0)
            warm_f32 = constp.tile([1, 1], F32, name="warm_f32")
            # also preloads the exp table on the Activation engine
            nc.scalar.activation(
                warm_f32[:], warm[0:1, 0:1], mybir.ActivationFunctionType.Exp
            )
            warm_ps = ps_p.tile([128, 128], F32, tag="proj")
            for i in range(64):
                nc.tensor.matmul(
                    warm_ps[:], warm[:], warm[:], start=(i == 0), stop=(i == 63)
                )
            # fine-grained tail keeps the PE clock ramped across the 2-4us
            # idle until the input DMA lands (~50 early matmuls otherwise run
            # at mid clock); 32-col granularity never blocks real work
            for i in range(48):
                nc.tensor.matmul(
                    warm_ps[0:32, 0:32],
                    warm[:, 0:32],
                    warm[:, 0:32],
                    start=(i == 0),
                    stop=(i == 47),
                )

            # ---- input DMA, split across both HW DGE queues ----
            wqpack_sb = constp.tile([128, 8, 128], F16, name="wqpack_sb")
            wrest_sb = constp.tile([128, 18, 128], F16, name="wrest_sb")
            nc.scalar.dma_start(
                wqpack_sb[:], wqpack_d.ap().rearrange("p (c h) -> p c h", c=8)
            )

            xt_sb = persist.tile([128, NRB, NDC, RB], F16, name="xt_sb")

            def load_slice(eng, g, c0=0, c1=NDC):
                eng.dma_start(
                    xt_sb[:, g, c0:c1, :],
                    xt_d.ap()[
                        :, (g * NDC + c0) * RB : (g * NDC + c1) * RB
                    ].rearrange("p (c s) -> p c s", c=c1 - c0),
                )

            # slice 0 in halves split ACROSS the two queues (they complete in
            # parallel, so the first projection starts ~2us sooner); remaining
            # slices interleaved so each arrives well before its consumer
            load_slice(nc.sync, 0, 0, NDC // 2)
            load_slice(nc.scalar, 0, NDC // 2, NDC)
            nc.scalar.dma_start(
                wrest_sb[:], wrest_d.ap().rearrange("p (c h) -> p c h", c=18)
            )
            load_slice(nc.sync, 1)
            load_slice(nc.scalar, 2)
            load_slice(nc.sync, 3)
            load_slice(nc.scalar, 4)
            load_slice(nc.sync, 5)
            load_slice(nc.scalar, 6)
            load_slice(nc.sync, 7)

            def w(name, dc):
                if name == "wq":
                    return wqpack_sb[:, dc, :]
                return wrest_sb[:, (0 if name == "wk" else 8) + dc, :]

            ident = wrest_sb[:, 16, :]
            ones = wrest_sb[:, 17, 0:1]

            # ---- persistent activations ----
            qt_sb = persist.tile([128, S], F16, name="qt_sb")  # [h, q] all q
            kt_sb = persist.tile([128, SK], F16, name="kt_sb")  # [h, k] own
            v_sb = persist.tile([128, NKB, H], F16, name="v_sb")  # own keys
            vt_sb = persist.tile([128, SK], F16, name="vt_sb")  # staging

            def project(wname, dst_sb, rb, pool, tag, width):
                """One 512-row projection block through one PSUM bank."""
                ps = pool.tile([128, width], F32, tag=tag)
                for dc in range(NDC):
                    nc.tensor.matmul(
                        ps[:, 0:RB],
                        w(wname, dc),
                        xt_sb[:, rb, dc, :],
                        start=(dc == 0),
                        stop=(dc == NDC - 1),
                    )
                nc.vector.tensor_copy(dst_sb[:, rb * RB : (rb + 1) * RB], ps[:, 0:RB])

            def v_transpose(g):
                v_ps = ps_p.tile([128, RB], F16, tag="proj")
                for s in range(4):
                    nc.tensor.transpose(
                        v_ps[:, s * 128 : (s + 1) * 128],
                        vt_sb[:, g * RB + s * 128 : g * RB + (s + 1) * 128],
                        ident,
                    )
                nc.vector.tensor_copy(
                    v_sb[:, g * 4 : (g + 1) * 4, :].rearrange("p a b -> p (a b)"),
                    v_ps[:, 0 : 4 * H],
                )

            # Front: only what attention chunk 0 needs immediately,
            # accumulated in parallel on idle attention PSUM slots.
            # Minimal front: score(0) only needs qt blocks 0,1 and kt block 0.
            # Everything else (v0 included) is interleaved into the attention
            # chunks, whose exp stream then starts ~4us earlier; chunk 0 lags
            # its AV accumulation by 3 kb to give the v0 chain time.
            project("wq", qt_sb, 0, ps_s, "st", QC)
            project("wq", qt_sb, 1, ps_o, "l", 512)
            project("wk", kt_sb, 0, ps_s, "st", QC)

            # deferred[qcidx][kb] emitted after score(kb).
            deferred = {qc: {} for qc in range(NQC)}

            def defer(qc, kb, fn):
                deferred[qc].setdefault(kb, []).append(fn)

            # chunk 0: kt block g ready 3 kb before score(4g); v block g
            # ready a kb before the (4-lagged) av(4g)
            defer(0, 0, lambda: project("wv", vt_sb, 0, ps_o, "outT0", 512))
            defer(0, 1, lambda: project("wk", kt_sb, 1, ps_p, "proj", RB))
            defer(0, 2, lambda: v_transpose(0))
            defer(0, 3, lambda: project("wv", vt_sb, 1, ps_p, "proj", RB))
            defer(0, 4, lambda: project("wk", kt_sb, 2, ps_p, "proj", RB))
            defer(0, 5, lambda: v_transpose(1))
            defer(0, 6, lambda: project("wv", vt_sb, 2, ps_p, "proj", RB))
            defer(0, 7, lambda: project("wk", kt_sb, 3, ps_p, "proj", RB))
            defer(0, 8, lambda: v_transpose(2))
            defer(0, 9, lambda: project("wv", vt_sb, 3, ps_p, "proj", RB))
            defer(0, 10, lambda: v_transpose(3))
            # qt blocks 2-7, needed one chunk ahead
            defer(0, 11, lambda: project("wq", qt_sb, 2, ps_p, "proj", RB))
            defer(0, 13, lambda: project("wq", qt_sb, 3, ps_p, "proj", RB))
            defer(1, 2, lambda: project("wq", qt_sb, 4, ps_p, "proj", RB))
            defer(1, 8, lambda: project("wq", qt_sb, 5, ps_p, "proj", RB))
            defer(2, 2, lambda: project("wq", qt_sb, 6, ps_p, "proj", RB))
            defer(2, 8, lambda: project("wq", qt_sb, 7, ps_p, "proj", RB))

            # ---- attention (software-pipelined by one kb) ----
            for qcidx in range(NQC):
                # per-half PSUM tiles: the h0 evacuation only waits on h0's
                # accumulation, overlapping the h1 matmuls at chunk end.
                # Created lazily at the first av so the chunk-0 v0-projection
                # (which borrows the outT0 bank) precedes them in rotation.
                outT_lazy = []

                def outT_ps(h):
                    if not outT_lazy:
                        outT_lazy.append(
                            ps_o.tile([128, 512], F32, tag="outT0", name="outT_ps0")
                        )
                        outT_lazy.append(
                            ps_o.tile([128, 512], F32, tag="outT1", name="outT_ps1")
                        )
                    return outT_lazy[h]
                # both 512-halves of l packed into ONE psum bank (partition 0
                # and partition 32 via tile_position col 32)
                l_ps = ps_o.tile([64, 512], F32, tag="l")
                at_tiles = {}
                pair_tiles = {}
                quad_tiles = {}

                def score(kb):
                    st_ps = ps_s.tile([128, QC], F32, tag="st")
                    for h in range(QC // 512):
                        nc.tensor.matmul(
                            st_ps[:, h * 512 : (h + 1) * 512],
                            kt_sb[:, kb * 128 : (kb + 1) * 128],
                            qt_sb[
                                :, qcidx * QC + h * 512 : qcidx * QC + (h + 1) * 512
                            ],
                            start=True,
                            stop=True,
                        )
                    at_sb = attn_pool.tile([128, QC], F16, tag="at")
                    nc.scalar.activation(
                        at_sb[:],
                        st_ps[:],
                        mybir.ActivationFunctionType.Exp,
                        scale=scale,
                    )
                    at_tiles[kb] = at_sb

                def accum_av(kb):
                    at_sb = at_tiles[kb]
                    for h in range(QC // 512):
                        nc.tensor.matmul(
                            outT_ps(h)[:],
                            v_sb[:, kb, :],
                            at_sb[:, h * 512 : (h + 1) * 512],
                            start=(kb == 0),
                            stop=(kb == NKB - 1),
                        )

                def pair_add(p):
                    pair = attn_pool.tile([128, QC], F16, tag="pair", bufs=3)
                    nc.vector.tensor_add(pair[:], at_tiles[2 * p][:], at_tiles[2 * p + 1][:])
                    pair_tiles[p] = pair

                def quad_add(qd):
                    quad = attn_pool.tile([128, QC], F16, tag="quad", bufs=3)
                    nc.vector.tensor_add(
                        quad[:], pair_tiles.pop(2 * qd)[:], pair_tiles.pop(2 * qd + 1)[:]
                    )
                    quad_tiles[qd] = quad

                def tree_add(dst, a, b):
                    t_ = attn_pool.tile([128, QC], F16, tag="lsum", bufs=3, name="lsum")
                    nc.vector.tensor_add(t_[:], a, b)
                    quad_tiles[dst] = t_

                # softmax-denominator matmul parts: the DVE tree folds
                # kb 0-13 into one "dec" tile; kb 14/15 feed the ones-matmul
                # DIRECTLY (dep = exp output only), and all three parts are
                # emitted after av15 so the in-order PE queue never waits on
                # the DVE chain
                NPART = 3

                def accum_l(part, src):
                    for h in range(QC // 512):
                        nc.tensor.matmul(
                            l_ps[h * 32 : h * 32 + 1, :],
                            ones,
                            src[:, h * 512 : (h + 1) * 512],
                            start=(part == 0),
                            stop=(part == NPART - 1),
                            tile_position=(0, h * 32),
                        )

                LAG = 4 if qcidx == 0 else 1
                score(0)
                for fn in deferred[qcidx].get(0, []):
                    fn()
                for kb in range(1, NKB):
                    score(kb)
                    for fn in deferred[qcidx].get(kb, []):
                        fn()
                    if kb % 2 == 1 and kb < 15:
                        pair_add((kb - 1) // 2)
                    if kb in (5, 9, 13):
                        quad_add((kb - 5) // 4)
                    if kb == 10:
                        tree_add("oct", quad_tiles.pop(0)[:], quad_tiles.pop(1)[:])
                    if kb == 14:
                        tree_add("hex", quad_tiles.pop("oct")[:], quad_tiles.pop(2)[:])
                    if kb == 15:
                        tree_add("dec", quad_tiles.pop("hex")[:], pair_tiles.pop(6)[:])
                    if kb >= LAG:
                        accum_av(kb - LAG)
                for j in range(NKB - LAG, NKB):
                    accum_av(j)
                accum_l(0, quad_tiles.pop("dec")[:])
                accum_l(1, at_tiles[14][:])
                accum_l(2, at_tiles[15][:])

                # evacuate: l via DVE copies mid-stream (the Activation
                # engine paces the attention phase), but via the Activation
                # engine on the last chunk so l and outT drain in parallel
                def evac_l():
                    l_sb = fin_pool.tile([1, QC], F16, tag="l_sb", name="l_sb")
                    if qcidx == NQC - 1:
                        nc.scalar.copy(l_sb[:, 0:512], l_ps[0:1, :])
                        nc.scalar.copy(l_sb[:, 512:1024], l_ps[32:33, :])
                    else:
                        nc.vector.tensor_copy(l_sb[:, 0:512], l_ps[0:1, :])
                        nc.vector.tensor_copy(l_sb[:, 512:1024], l_ps[32:33, :])
                    nc.sync.dma_start(
                        l_d.ap()[:, qcidx * QC : (qcidx + 1) * QC], l_sb[:]
                    )

                if qcidx != NQC - 1:
                    evac_l()
                for hh in range(2):
                    outT_sb = fin_pool.tile([128, 512], F16, tag="outT_sb", bufs=2)
                    if qcidx == NQC - 1 and hh == 1:
                        # last chunk: h1 copy on the (now idle) Activation
                        # engine, in parallel with the DVE h0 copy
                        nc.scalar.copy(outT_sb[:], outT_ps(hh)[:])
                    else:
                        nc.vector.tensor_copy(outT_sb[:], outT_ps(hh)[:])
                    nc.sync.dma_start(
                        outT_d.ap()[
                            :, qcidx * QC + hh * 512 : qcidx * QC + (hh + 1) * 512
                        ],
                        outT_sb[:],
                    )
                if qcidx == NQC - 1:
                    evac_l()

    nc.compile()
    return nc


def _get_nc():
    if "nc" not in _CACHE:
        _CACHE["nc"] = build_nc()
    return _CACHE["nc"]


def make_in_maps(inputs, Wq, Wk, Wv):
    inputs = np.asarray(inputs, dtype=np.float32)

    def _swz(wmat):
        # chunk c holds rows c*128..c*128+127: out[p, c, h] = W[c*128+p, h]
        w16 = np.asarray(wmat, dtype=np.float16)  # [D, H]
        return w16.reshape(NDC, 128, H).transpose(1, 0, 2)

    wqpack = np.ascontiguousarray(_swz(Wq).reshape(128, 8 * 128))
    wrest = np.zeros((128, 18, 128), dtype=np.float16)
    wrest[:, 0:8, :] = _swz(Wk)
    wrest[:, 8:16, :] = _swz(Wv)
    wrest[:, 16, :] = np.eye(128, dtype=np.float16)
    wrest[:, 17, 0] = 1.0
    wrest_flat = np.ascontiguousarray(wrest.reshape(128, 18 * 128))

    in_maps = []
    for c in range(NCORES):
        b, kh = divmod(c, 2)
        xb = inputs[b]
        # own key-half rows first; queries follow the same permutation
        xk = np.concatenate(
            [xb[kh * SK : (kh + 1) * SK], xb[(1 - kh) * SK : (2 - kh) * SK]], axis=0
        )
        x16 = xk.astype(np.float16)  # [S, D]
        # xt[p, g, c, s] = xk[g*RB+s, c*128+p]: slice-major, 8KB contiguous
        # per (partition, slice)
        xt = np.ascontiguousarray(
            x16.reshape(NRB, RB, NDC, 128).transpose(3, 0, 2, 1)
        ).reshape(128, NRB * NDC * RB)
        in_maps.append({"xt": xt, "wqpack": wqpack, "wrest": wrest_flat})
    return in_maps


def kernel(inputs, Wq, Wk, Wv):
    nc = _get_nc()
    in_maps = make_in_maps(inputs, Wq, Wk, Wv)

    res = run_bass_kernel_spmd(nc, in_maps, core_ids=list(range(NCORES)))

    out = np.empty((B, S, H), dtype=np.float32)
    for b in range(B):
        num = np.zeros((H, S), dtype=np.float32)
        den = np.zeros((1, S), dtype=np.float32)
        for kh in range(2):
            c = 2 * b + kh
            outT = res.results[c]["outT"].astype(np.float32)  # [H, S], permuted q
            l = res.results[c]["l"].astype(np.float32)  # [1, S]
            # queries were ordered [kh-half, other-half]; map back
            perm = np.concatenate(
                [
                    np.arange(kh * SK, (kh + 1) * SK),
                    np.arange((1 - kh) * SK, (2 - kh) * SK),
                ]
            )
            num[:, perm] += outT
            den[:, perm] += l
        out[b] = (num / den).T
    return out
